# revision 9
# baseline (speedup 1.0000x reference)
"""BiLSTM + vocab projection + log_softmax Trainium2 kernel.

Strategy (8 NeuronCores, batch-parallel):
  - Shard batch B=64 -> 8 rows per core. LSTM recurrence is per-batch-row,
    so each core runs the full fwd+bwd LSTM over S=128 for its 8 rows.
  - State kept transposed: H^T [32 h-part, 8 b], C^T [32 c-part, 8 b].
    Scalar gates (f,i,o) are broadcast across the 32 c-partitions by
    replicating the gate weight column 32x in the stationary matmul operand,
    so gate*state products are plain elementwise DVE ops (no partition
    broadcasts needed).
  - The per-step H^T write goes directly into a transposed H table
    HtabT [65, 1024] (rows 0:32 fwd h, 32:64 bwd h, row 64 = ones for the
    output bias; col = 8*s + b). Projection lhsT tiles are direct slices.
  - Projection: logits = Hcat @ Wout + bout over V=50257, log_softmax over V.
    Wout_ext [65, V] (row 64 = bout) is loaded ONCE into SBUF as bf16
    (100.5 KB/partition) at kernel start, overlapping the LSTM phase; the
    projection then streams it from SBUF, so HBM traffic is just the
    6.5 MB load + the 206 MB output store (the memory floor).
    Pass 1 computes exp(logits) per chunk via ACT (no max subtraction
    needed: |logits| <= ~12, fp32 safe), accumulating the row sums via
    accum_out and SAVING the exp values for the first VCACHE vocab columns
    as bf16 in SBUF. Pass 2 emits logits - ln(sum) two ways, splitting the
    work across engines so everything hides under the store DMA:
      * cached columns:   ACT  ln(exp_saved * (1/sum))   (no matmul)
      * uncached columns: PE recompute matmul + DVE subtract ln(sum)
    Pass 2 of tile r is interleaved with pass 1 of tile r+1 so the store
    stream never starves.
"""

import numpy as np

V = 50257
E = 128
HS = 32
S = 128
B = 64
NCORES = 8
BL = B // NCORES          # 8 batch rows per core
ROWS = S * BL             # 1024 output rows per core
SUB = 2048                # exp/affine granularity, main phase (4 PSUM banks)
SUBO = 1536               # chunk size while overlapped with the LSTM (3 banks)
VT = 512                  # matmul N tile (one PSUM bank of fp32)
VCACHE = 12288            # leading vocab cols whose exp is cached in SBUF
LNW = 2048                # ln-pass / store width for cached cols
OSTEP = 80                # LSTM step at which tile-3 projection can start


def _ceil_div(a, b):
    return (a + b - 1) // b


def _build(nc, tile, mybir, bass, phases=("pre", "lstm", "proj")):
    from concourse.masks import make_identity

    f32 = mybir.dt.float32
    bf16 = mybir.dt.bfloat16
    AF = mybir.ActivationFunctionType
    OP = mybir.AluOpType

    # ---------------- DRAM I/O ----------------
    idx_d = nc.dram_tensor("idx", [128, 8], mybir.dt.int32, kind="ExternalInput")
    lut_d = nc.dram_tensor("lut", [V, E], f32, kind="ExternalInput")
    wx_d = nc.dram_tensor("wx", [128, 256], f32, kind="ExternalInput")
    wh_d = nc.dram_tensor("wh", [64, 256], f32, kind="ExternalInput")
    bt_d = nc.dram_tensor("bt", [64, 4], f32, kind="ExternalInput")
    ih_d = nc.dram_tensor("ih", [64, 8], f32, kind="ExternalInput")
    ic_d = nc.dram_tensor("ic", [64, 8], f32, kind="ExternalInput")
    wo_d = nc.dram_tensor("wo", [65, V], bf16, kind="ExternalInput")
    out_d = nc.dram_tensor("out", [ROWS, V], f32, kind="ExternalOutput")

    nsub = _ceil_div(V, SUB)            # 25 chunks

    with tile.TileContext(nc) as tc:
        with tc.tile_pool(name="persist", bufs=1) as pp:
            # persistent SBUF state
            wo_sb = pp.tile([65, V], bf16)       # resident Wout (+bias row)
            idx_sb = pp.tile([128, 8], mybir.dt.int32)
            wh_sb = pp.tile([64, 256], f32)      # 4x block-diag [whf_g|whb_g]
            bt_sb = pp.tile([64, 4], f32)
            wx_sb = pp.tile([128, 256], f32)
            id128 = pp.tile([128, 128], f32)
            id64 = pp.tile([64, 64], f32)
            # time-indexed H table: col-block u = state READ at step u.
            # rows 0:32 fwd (== slot order), rows 32:64 bwd (slot 127-u),
            # row 64 = ones for the output bias.
            htab = pp.tile([65, 8 * S], f32)
            cst = pp.tile([64, 8], f32)          # C^T state (fwd rows 0:32, bwd 32:64)
            htabr = pp.tile([65, 8 * S], bf16)   # bf16 copy for the projection
            cache = pp.tile([128, VCACHE], bf16) # saved exp(logits), cols 0:VCACHE
            logz = pp.tile([128, 8], f32)        # per row-tile log-partition
            rsum = pp.tile([128, 8], f32)        # per row-tile 1/sum(exp)
            parts = [pp.tile([128, 40], f32, name=f"part{r}") for r in range(8)]

            # Wout load first: no deps, overlaps the whole pre+LSTM phase.
            nc.sync.dma_start(out=wo_sb[:], in_=wo_d[:])
            nc.sync.dma_start(out=idx_sb[:], in_=idx_d[:])
            nc.sync.dma_start(out=wh_sb[:], in_=wh_d[:])
            nc.sync.dma_start(out=bt_sb[:], in_=bt_d[:])
            nc.sync.dma_start(out=wx_sb[:], in_=wx_d[:])
            nc.gpsimd.memset(htab[64:65, :], 1.0)
            make_identity(nc, id128[:])
            make_identity(nc, id64[:])
            # initial states: both directions read col-block 0 at step 0
            nc.sync.dma_start(out=htab[0:64, 0:8], in_=ih_d[:])
            nc.sync.dma_start(out=cst[:], in_=ic_d[:])

            # ---------------- embedding gather + X^T + XW tables ----------------
            if "pre" not in phases:
                return nc
            nc.gpsimd.memset(htabr[64:65, :], 1.0)   # ones row (no htab dep)

            with tc.tile_pool(name="stg", bufs=3) as sp, \
                 tc.tile_pool(name="xw", bufs=1) as xwp:
              xwall = xwp.tile([64, 32 * S], f32)    # per-step gate pre-acts
              with tc.tile_pool(name="xtb", bufs=1) as xtp, \
                   tc.tile_pool(name="pre", bufs=2) as gp, \
                   tc.tile_pool(name="prepsum", bufs=2, space="PSUM") as gpp:
                xt = xtp.tile([128, ROWS], f32)      # X^T (E on partitions)
                for r in range(8):
                    xg = gp.tile([128, 128], f32, tag="xg", name="xg")
                    nc.gpsimd.indirect_dma_start(
                        out=xg[:],
                        out_offset=None,
                        in_=lut_d[:],
                        in_offset=bass.IndirectOffsetOnAxis(
                            ap=idx_sb[:, r:r + 1], axis=0),
                    )
                    xtp_t = gpp.tile([128, 128], f32, tag="xtp", name="xtp")
                    nc.tensor.transpose(out=xtp_t[:], in_=xg[:], identity=id128[:])
                    nc.vector.tensor_copy(out=xt[:, 128 * r:128 * (r + 1)], in_=xtp_t[:])

                # XW tables: fwd rows hold slot u, bwd rows slot 127-u (the
                # input the bwd cell consumes at step u) via a reversed AP.
                xw_v = xwall[:, :].rearrange("p (s g) -> p s g", g=32)
                xw_r = xw_v[:, ::-1, :]
                for d in range(2):
                    L = 32 * d
                    for g in range(4):
                        for c in range(2):
                            xwp_t = gpp.tile([64, 512], f32, tag="xwp", name="xwp")
                            nc.tensor.matmul(
                                out=xwp_t[L:L + 32, :],
                                lhsT=wx_sb[:, 128 * d + 32 * g:128 * d + 32 * (g + 1)],
                                rhs=xt[:, 512 * c:512 * (c + 1)],
                                start=True, stop=True,
                            )
                            ov = xw_v if d == 0 else xw_r
                            nc.vector.tensor_scalar(
                                out=ov[L:L + 32, 64 * c:64 * (c + 1), 8 * g:8 * (g + 1)],
                                in0=xwp_t[L:L + 32, :].rearrange("p (s b) -> p s b", b=8),
                                scalar1=bt_sb[L:L + 32, g:g + 1],
                                scalar2=None,
                                op0=OP.add,
                            )

              if "lstm" not in phases:
                return nc

              # ---------- shared projection helpers (both phases) ----------
              def copy_htabr(r):
                  # fwd rows are slot==time ordered; bwd rows need reversal
                  nc.vector.tensor_copy(
                      out=htabr[0:32, 128 * r:128 * (r + 1)],
                      in_=htab[0:32, 128 * r:128 * (r + 1)])
                  hb_o = htabr[32:64, :].rearrange("p (s b) -> p s b", b=8)
                  hb_i = htab[32:64, :].rearrange("p (s b) -> p s b", b=8)
                  nc.vector.tensor_copy(
                      out=hb_o[:, 16 * r:16 * (r + 1), :],
                      in_=hb_i[:, ::-1, :][:, 16 * r:16 * (r + 1), :])

              def p1_chunk(r, c, size, pool):
                  s0 = c * size
                  ss = min(size, V - s0)
                  pj = pool.tile([128, size], f32, tag="pj", name="pj")
                  for v in range(_ceil_div(ss, VT)):
                      v0 = s0 + v * VT
                      vs = min(VT, V - v0)
                      nc.tensor.matmul(
                          out=pj[:, VT * v:VT * v + vs],
                          lhsT=htabr[:, 128 * r:128 * (r + 1)],
                          rhs=wo_sb[:, v0:v0 + vs],
                          start=True, stop=True,
                      )
                  if s0 + ss <= VCACHE:
                      eout = cache[:, s0:s0 + ss]
                  else:
                      scr = sp.tile([128, SUB], bf16, tag="scr", name="scr")
                      eout = scr[:, :ss]
                  nc.scalar.activation(
                      eout, pj[:, :ss], AF.Exp,
                      accum_out=parts[r][:, c:c + 1])

              def p1_finish(r, nsubr):
                  ssum = sp.tile([128, 1], f32, tag="ssum", name="ssum")
                  nc.vector.tensor_reduce(
                      out=ssum[:], in_=parts[r][:, :nsubr],
                      axis=mybir.AxisListType.X, op=OP.add)
                  nc.scalar.activation(logz[:, r:r + 1], ssum[:], AF.Ln)
                  nc.vector.reciprocal(rsum[:, r:r + 1], ssum[:])

              def p2_ln(r, j):
                  # cached cols: out = ln(exp_saved * (1/sum))
                  s0 = j * LNW
                  stg = sp.tile([128, SUB], f32, tag="stg", name="stg", bufs=4)
                  nc.scalar.activation(
                      stg[:, :LNW], cache[:, s0:s0 + LNW], AF.Ln,
                      scale=rsum[:, r:r + 1])
                  eng = nc.sync if j % 2 == 0 else nc.scalar
                  eng.dma_start(
                      out=out_d[128 * r:128 * (r + 1), s0:s0 + LNW],
                      in_=stg[:, :LNW])

              def p2_mm(r, j, size, pool):
                  # uncached cols: recompute logits, subtract ln(sum)
                  s0 = VCACHE + j * size
                  ss = min(size, V - s0)
                  pj2 = pool.tile([128, size], f32, tag="pj", name="pj2")
                  for v in range(_ceil_div(ss, VT)):
                      v0 = s0 + v * VT
                      vs = min(VT, V - v0)
                      nc.tensor.matmul(
                          out=pj2[:, VT * v:VT * v + vs],
                          lhsT=htabr[:, 128 * r:128 * (r + 1)],
                          rhs=wo_sb[:, v0:v0 + vs],
                          start=True, stop=True,
                      )
                  stg = sp.tile([128, SUB], f32, tag="stg", name="stg", bufs=4)
                  nc.vector.tensor_scalar(
                      out=stg[:, :ss], in0=pj2[:, :ss],
                      scalar1=logz[:, r:r + 1], scalar2=None,
                      op0=OP.subtract)
                  eng = nc.sync if j % 2 == 0 else nc.scalar
                  eng.dma_start(
                      out=out_d[128 * r:128 * (r + 1), s0:s0 + ss],
                      in_=stg[:, :ss])

              NLN = VCACHE // LNW                 # 6 ln chunks per tile
              NSO = _ceil_div(V, SUBO)            # 33 overlap p1 chunks
              NRO = _ceil_div(V - VCACHE, SUBO)   # 25 overlap rec chunks
              NSM = _ceil_div(V, SUB)             # 25 main p1 chunks
              NRM = _ceil_div(V - VCACHE, SUB)    # 19 main rec chunks

              # ------------- LSTM + overlapped projection start -------------
              with tc.tile_pool(name="lstm", bufs=3) as lp, \
                   tc.tile_pool(name="lstmpsum", bufs=2, space="PSUM") as lpp, \
                   tc.tile_pool(name="ovlpsum", bufs=2, space="PSUM") as jpo:

                def step(t):
                    gall = lpp.tile([64, 32], f32, tag="gall", name="gall")
                    nc.tensor.matmul(
                        out=gall[:],
                        lhsT=id64[:],
                        rhs=xwall[:, 32 * t:32 * (t + 1)],
                        start=True, stop=False,
                    )
                    for g in range(4):
                        nc.tensor.matmul(
                            out=gall[:, 8 * g:8 * (g + 1)],
                            lhsT=wh_sb[:, 64 * g:64 * (g + 1)],
                            rhs=htab[0:64, 8 * t:8 * (t + 1)],
                            start=False, stop=(g == 3),
                            skip_group_check=True,
                        )
                    sall = lp.tile([64, 32], f32, tag="sall", name="sall")
                    nc.scalar.activation(sall[:], gall[:], AF.Sigmoid)
                    cts = lp.tile([64, 8], f32, tag="cts", name="cts")
                    nc.vector.tensor_scalar(
                        out=cts[:], in0=sall[:, 24:32],
                        scalar1=2.0, scalar2=-1.0, op0=OP.mult, op1=OP.add)
                    t2 = lp.tile([64, 8], f32, tag="t2", name="t2")
                    nc.vector.tensor_tensor(out=t2[:], in0=sall[:, 8:16], in1=cts[:], op=OP.mult)
                    t3 = lp.tile([64, 8], f32, tag="t3", name="t3")
                    nc.vector.tensor_tensor(out=t3[:], in0=sall[:, 0:8], in1=cst[:], op=OP.mult)
                    nc.vector.tensor_tensor(out=cst[:], in0=t2[:], in1=t3[:], op=OP.add)
                    th = lp.tile([64, 8], f32, tag="th", name="th")
                    nc.scalar.activation(th[:], cst[:], AF.Tanh)
                    nc.vector.tensor_tensor(
                        out=htab[0:64, 8 * (t + 1):8 * (t + 2)],
                        in0=th[:], in1=sall[:, 16:24], op=OP.mult)

                for t in range(0, OSTEP):
                    step(t)

                # phase A: steps 80..95 || p1(3)
                copy_htabr(3)
                done = 0
                for i, t in enumerate(range(OSTEP, OSTEP + 16)):
                    step(t)
                    tgt = (i + 1) * NSO // 16
                    while done < tgt:
                        p1_chunk(3, done, SUBO, jpo)
                        done += 1
                p1_finish(3, NSO)

                # phase B: steps 96..111 || p2(3) + p1(4)
                copy_htabr(4)
                for j in range(NLN):      # all cache reads of tile 3 BEFORE
                    p2_ln(3, j)           # tile 4's exp overwrites the cache
                emitted = [0, 0]          # rec(3), p1(4)
                for i, t in enumerate(range(OSTEP + 16, OSTEP + 32)):
                    step(t)
                    for k, total in ((0, NRO), (1, NSO)):
                        tgt = (i + 1) * total // 16
                        while emitted[k] < tgt:
                            jj = emitted[k]
                            if k == 0:
                                p2_mm(3, jj, SUBO, jpo)
                            else:
                                p1_chunk(4, jj, SUBO, jpo)
                            emitted[k] += 1
                p1_finish(4, NSO)

                # phase C: steps 112..126 || p2(4) + p1(2)
                copy_htabr(2)
                for j in range(NLN):
                    p2_ln(4, j)
                emitted = [0, 0]
                nrem = (S - 1) - (OSTEP + 32)
                for i, t in enumerate(range(OSTEP + 32, S - 1)):
                    step(t)
                    for k, total in ((0, NRO), (1, NSO)):
                        tgt = (i + 1) * total // nrem
                        while emitted[k] < tgt:
                            jj = emitted[k]
                            if k == 0:
                                p2_mm(4, jj, SUBO, jpo)
                            else:
                                p1_chunk(2, jj, SUBO, jpo)
                            emitted[k] += 1
                p1_finish(2, NSO)

              # xwall + lstm pools closed; main projection phase
              if "proj" not in phases:
                return nc
              with tc.tile_pool(name="mainpsum", bufs=2, space="PSUM") as jpm:
                PAIRS = [(5, 2), (1, 5), (6, 1), (0, 6), (7, 0), (None, 7)]
                for r1, r2 in PAIRS:
                    if r1 is not None:
                        copy_htabr(r1)
                    for j in range(NLN):
                        p2_ln(r2, j)
                    for c in range(NSM):
                        for j in range(c * NRM // NSM, (c + 1) * NRM // NSM):
                            p2_mm(r2, j, SUB, jpm)
                        if r1 is not None:
                            p1_chunk(r1, c, SUB, jpm)
                    if r1 is not None:
                        p1_finish(r1, NSM)
    return nc


def _prep_shared(inputs, bf16_np):
    """Build the numpy operands shared by all cores."""
    f = lambda k: np.asarray(inputs[k], np.float32)
    Wf1, Wi1, WC1, Wo1 = f("Wf1"), f("Wi1"), f("WC1"), f("Wo1")
    Wf2, Wi2, WC2, Wo2 = f("Wf2"), f("Wi2"), f("WC2"), f("Wo2")

    def rep(w):  # [128,1] -> [128,32] replicated
        return np.tile(w, (1, 32)).astype(np.float32)

    wx = np.concatenate(
        [rep(Wf1[HS:, :]), rep(Wi1[HS:, :]), rep(Wo1[HS:, :]), 2.0 * WC1[HS:, :],
         rep(Wf2[HS:, :]), rep(Wi2[HS:, :]), rep(Wo2[HS:, :]), 2.0 * WC2[HS:, :]],
        axis=1)  # [128, 256]
    # block-diag per gate: [64, 64] block g = [[whf_g, 0], [0, whb_g]]
    whf = np.concatenate(
        [rep(Wf1[:HS, :]), rep(Wi1[:HS, :]), rep(Wo1[:HS, :]), 2.0 * WC1[:HS, :]], axis=1)
    whb = np.concatenate(
        [rep(Wf2[:HS, :]), rep(Wi2[:HS, :]), rep(Wo2[:HS, :]), 2.0 * WC2[:HS, :]], axis=1)
    wh = np.zeros((64, 256), np.float32)
    for g in range(4):
        wh[0:32, 64 * g:64 * g + 32] = whf[:, 32 * g:32 * (g + 1)]
        wh[32:64, 64 * g + 32:64 * (g + 1)] = whb[:, 32 * g:32 * (g + 1)]

    bt = np.zeros((64, 4), np.float32)
    for col, (b1, b2) in enumerate(
            [("bf1", "bf2"), ("bi1", "bi2"), ("bo1", "bo2")]):
        bt[0:32, col] = f(b1)[0]
        bt[32:64, col] = f(b2)[0]
    bt[0:32, 3] = 2.0 * f("bC1")
    bt[32:64, 3] = 2.0 * f("bC2")

    ih = np.zeros((64, 8), np.float32)
    ih[0:32] = np.tile(f("Hf")[:, None], (1, 8))
    ih[32:64] = np.tile(f("Hb")[:, None], (1, 8))
    ic = np.zeros((64, 8), np.float32)
    ic[0:32] = np.tile(f("Cf")[:, None], (1, 8))
    ic[32:64] = np.tile(f("Cb")[:, None], (1, 8))

    wo = np.zeros((65, V), np.float32)
    wo[0:64] = f("Wout")
    wo[64] = f("bout")

    lut = np.ascontiguousarray(f("lookup"))
    return dict(lut=lut, wx=np.ascontiguousarray(wx), wh=np.ascontiguousarray(wh),
                bt=bt, ih=ih, ic=ic,
                wo=np.ascontiguousarray(wo.astype(bf16_np)))


_TRACE = False          # set by test.py to capture an NTFF profile
_LAST_RESULT = None     # BassKernelResults from the most recent run


def kernel(**inputs):
    global _LAST_RESULT
    import concourse.bass as bass
    import concourse.mybir as mybir
    import concourse.tile as tile
    from concourse import bacc
    from concourse.bass_utils import run_bass_kernel_spmd

    nc = bacc.Bacc("TRN2", target_bir_lowering=False)
    _build(nc, tile, mybir, bass)
    nc.compile()

    shared = _prep_shared(inputs, mybir.dt.np(mybir.dt.bfloat16))
    ib = np.asarray(inputs["input_batch"]).astype(np.int32)  # [S, B]

    in_maps = []
    for k in range(NCORES):
        idx_flat = np.ascontiguousarray(ib[:, BL * k:BL * (k + 1)]).reshape(ROWS)
        idx_t = np.ascontiguousarray(idx_flat.reshape(8, 128).T)  # [128, 8]
        in_maps.append(dict(idx=idx_t, **shared))

    res = run_bass_kernel_spmd(nc, in_maps, core_ids=list(range(NCORES)),
                               trace=_TRACE)
    _LAST_RESULT = res
    outs = [r["out"].reshape(S, BL, V) for r in res.results]
    return np.concatenate(outs, axis=1)


if __name__ == "__main__":
    import concourse.bass as bass
    import concourse.mybir as mybir
    import concourse.tile as tile
    from concourse import bacc

    nc = bacc.Bacc("TRN2", target_bir_lowering=False)
    _build(nc, tile, mybir, bass)
    nc.compile()
    print("build ok")


# revision 10
# speedup vs baseline: 1.2190x; 1.2190x over previous
"""BiLSTM + vocab projection + log_softmax Trainium2 kernel.

Strategy (8 NeuronCores, batch-parallel):
  - Shard batch B=64 -> 8 rows per core. LSTM recurrence is per-batch-row,
    so each core runs the full fwd+bwd LSTM over S=128 for its 8 rows.
  - State kept transposed: H^T [32 h-part, 8 b], C^T [32 c-part, 8 b].
    Scalar gates (f,i,o) are broadcast across the 32 c-partitions by
    replicating the gate weight column 32x in the stationary matmul operand,
    so gate*state products are plain elementwise DVE ops (no partition
    broadcasts needed).
  - The per-step H^T write goes directly into a transposed H table
    HtabT [65, 1024] (rows 0:32 fwd h, 32:64 bwd h, row 64 = ones for the
    output bias; col = 8*s + b). Projection lhsT tiles are direct slices.
  - Projection: logits = Hcat @ Wout + bout over V=50257, log_softmax over V.
    Wout_ext [65, V] (row 64 = bout) is loaded ONCE into SBUF as bf16
    (100.5 KB/partition) at kernel start, overlapping the LSTM phase; the
    projection then streams it from SBUF, so HBM traffic is just the
    6.5 MB load + the 206 MB output store (the memory floor).
    Pass 1 computes exp(logits) per chunk via ACT (no max subtraction
    needed: |logits| <= ~12, fp32 safe), accumulating the row sums via
    accum_out and SAVING the exp values for the first VCACHE vocab columns
    as bf16 in SBUF. Pass 2 emits logits - ln(sum) two ways, splitting the
    work across engines so everything hides under the store DMA:
      * cached columns:   ACT  ln(exp_saved * (1/sum))   (no matmul)
      * uncached columns: PE recompute matmul + DVE subtract ln(sum)
    Pass 2 of tile r is interleaved with pass 1 of tile r+1 so the store
    stream never starves.
"""

import numpy as np

V = 50257
E = 128
HS = 32
S = 128
B = 64
NCORES = 8
BL = B // NCORES          # 8 batch rows per core
ROWS = S * BL             # 1024 output rows per core
SUB = 2048                # exp/affine granularity, main phase (4 PSUM banks)
SUBO = 1536               # chunk size while overlapped with the LSTM (3 banks)
VT = 512                  # matmul N tile (one PSUM bank of fp32)
VCACHE = 12288            # leading vocab cols whose exp is cached in SBUF
LNW = 2048                # ln-pass / store width for cached cols
OSTEP = 80                # LSTM step at which tile-3 projection can start


def _ceil_div(a, b):
    return (a + b - 1) // b


def _build(nc, tile, mybir, bass, phases=("pre", "lstm", "proj")):
    from concourse.masks import make_identity

    f32 = mybir.dt.float32
    bf16 = mybir.dt.bfloat16
    AF = mybir.ActivationFunctionType
    OP = mybir.AluOpType

    # ---------------- DRAM I/O ----------------
    idx_d = nc.dram_tensor("idx", [128, 8], mybir.dt.int32, kind="ExternalInput")
    lut_d = nc.dram_tensor("lut", [V, E], f32, kind="ExternalInput")
    wx_d = nc.dram_tensor("wx", [128, 256], f32, kind="ExternalInput")
    wh_d = nc.dram_tensor("wh", [64, 256], f32, kind="ExternalInput")
    bt_d = nc.dram_tensor("bt", [64, 4], f32, kind="ExternalInput")
    ih_d = nc.dram_tensor("ih", [64, 8], f32, kind="ExternalInput")
    ic_d = nc.dram_tensor("ic", [64, 8], f32, kind="ExternalInput")
    wo_d = nc.dram_tensor("wo", [65, V], bf16, kind="ExternalInput")
    out_d = nc.dram_tensor("out", [ROWS, V], f32, kind="ExternalOutput")

    nsub = _ceil_div(V, SUB)            # 25 chunks

    with tile.TileContext(nc) as tc:
        with tc.tile_pool(name="persist", bufs=1) as pp:
            # persistent SBUF state
            wo_sb = pp.tile([65, V], bf16)       # resident Wout (+bias row)
            idx_sb = pp.tile([128, 8], mybir.dt.int32)
            wh_sb = pp.tile([64, 256], f32)      # 4x block-diag [whf_g|whb_g]
            bt_sb = pp.tile([64, 4], f32)
            wx_sb = pp.tile([128, 256], f32)
            id128 = pp.tile([128, 128], f32)
            id64 = pp.tile([64, 64], f32)
            # time-indexed H table: col-block u = state READ at step u.
            # rows 0:32 fwd (== slot order), rows 32:64 bwd (slot 127-u),
            # row 64 = ones for the output bias.
            htab = pp.tile([65, 8 * S], f32)
            cst = pp.tile([64, 8], f32)          # C^T state (fwd rows 0:32, bwd 32:64)
            htabr = pp.tile([65, 8 * S], bf16)   # bf16 copy for the projection
            cache = pp.tile([128, VCACHE], bf16) # saved exp(logits), cols 0:VCACHE
            logz = pp.tile([128, 8], f32)        # per row-tile log-partition
            rsum = pp.tile([128, 8], f32)        # per row-tile 1/sum(exp)
            parts = [pp.tile([128, 40], f32, name=f"part{r}") for r in range(8)]

            # Wout load first: no deps, overlaps the whole pre+LSTM phase.
            nc.sync.dma_start(out=wo_sb[:], in_=wo_d[:])
            nc.sync.dma_start(out=idx_sb[:], in_=idx_d[:])
            nc.sync.dma_start(out=wh_sb[:], in_=wh_d[:])
            nc.sync.dma_start(out=bt_sb[:], in_=bt_d[:])
            nc.sync.dma_start(out=wx_sb[:], in_=wx_d[:])
            nc.gpsimd.memset(htab[64:65, :], 1.0)
            make_identity(nc, id128[:])
            make_identity(nc, id64[:])
            # initial states: both directions read col-block 0 at step 0
            nc.sync.dma_start(out=htab[0:64, 0:8], in_=ih_d[:])
            nc.sync.dma_start(out=cst[:], in_=ic_d[:])

            # ---------------- embedding gather + X^T + XW tables ----------------
            if "pre" not in phases:
                return nc
            nc.gpsimd.memset(htabr[64:65, :], 1.0)   # ones row (no htab dep)

            with tc.tile_pool(name="stg", bufs=3) as sp, \
                 tc.tile_pool(name="xw", bufs=1) as xwp:
              xwall = xwp.tile([64, 32 * S], f32)    # per-step gate pre-acts
              with tc.tile_pool(name="xtb", bufs=1) as xtp, \
                   tc.tile_pool(name="pre", bufs=2) as gp, \
                   tc.tile_pool(name="prepsum", bufs=2, space="PSUM") as gpp:
                xt = xtp.tile([128, ROWS], f32)      # X^T (E on partitions)
                for r in range(8):
                    xg = gp.tile([128, 128], f32, tag="xg", name="xg")
                    nc.gpsimd.indirect_dma_start(
                        out=xg[:],
                        out_offset=None,
                        in_=lut_d[:],
                        in_offset=bass.IndirectOffsetOnAxis(
                            ap=idx_sb[:, r:r + 1], axis=0),
                    )
                    xtp_t = gpp.tile([128, 128], f32, tag="xtp", name="xtp")
                    nc.tensor.transpose(out=xtp_t[:], in_=xg[:], identity=id128[:])
                    nc.vector.tensor_copy(out=xt[:, 128 * r:128 * (r + 1)], in_=xtp_t[:])

                # XW tables: fwd rows hold slot u, bwd rows slot 127-u (the
                # input the bwd cell consumes at step u) via a reversed AP.
                xw_v = xwall[:, :].rearrange("p (s g) -> p s g", g=32)
                xw_r = xw_v[:, ::-1, :]
                for d in range(2):
                    L = 32 * d
                    for g in range(4):
                        for c in range(2):
                            xwp_t = gpp.tile([64, 512], f32, tag="xwp", name="xwp")
                            nc.tensor.matmul(
                                out=xwp_t[L:L + 32, :],
                                lhsT=wx_sb[:, 128 * d + 32 * g:128 * d + 32 * (g + 1)],
                                rhs=xt[:, 512 * c:512 * (c + 1)],
                                start=True, stop=True,
                            )
                            ov = xw_v if d == 0 else xw_r
                            nc.vector.tensor_scalar(
                                out=ov[L:L + 32, 64 * c:64 * (c + 1), 8 * g:8 * (g + 1)],
                                in0=xwp_t[L:L + 32, :].rearrange("p (s b) -> p s b", b=8),
                                scalar1=bt_sb[L:L + 32, g:g + 1],
                                scalar2=None,
                                op0=OP.add,
                            )

              if "lstm" not in phases:
                return nc

              # ---------- shared projection helpers (both phases) ----------
              def copy_htabr(r):
                  # fwd rows are slot==time ordered; bwd rows need reversal
                  nc.vector.tensor_copy(
                      out=htabr[0:32, 128 * r:128 * (r + 1)],
                      in_=htab[0:32, 128 * r:128 * (r + 1)])
                  hb_o = htabr[32:64, :].rearrange("p (s b) -> p s b", b=8)
                  hb_i = htab[32:64, :].rearrange("p (s b) -> p s b", b=8)
                  nc.vector.tensor_copy(
                      out=hb_o[:, 16 * r:16 * (r + 1), :],
                      in_=hb_i[:, ::-1, :][:, 16 * r:16 * (r + 1), :])

              def p1_chunk(r, c, size, pool):
                  s0 = c * size
                  ss = min(size, V - s0)
                  pj = pool.tile([128, size], f32, tag="pj", name="pj")
                  for v in range(_ceil_div(ss, VT)):
                      v0 = s0 + v * VT
                      vs = min(VT, V - v0)
                      nc.tensor.matmul(
                          out=pj[:, VT * v:VT * v + vs],
                          lhsT=htabr[:, 128 * r:128 * (r + 1)],
                          rhs=wo_sb[:, v0:v0 + vs],
                          start=True, stop=True,
                      )
                  if s0 + ss <= VCACHE:
                      eout = cache[:, s0:s0 + ss]
                  else:
                      scr = sp.tile([128, SUB], bf16, tag="scr", name="scr")
                      eout = scr[:, :ss]
                  nc.scalar.activation(
                      eout, pj[:, :ss], AF.Exp,
                      accum_out=parts[r][:, c:c + 1])

              def p1_finish(r, nsubr):
                  ssum = sp.tile([128, 1], f32, tag="ssum", name="ssum")
                  nc.vector.tensor_reduce(
                      out=ssum[:], in_=parts[r][:, :nsubr],
                      axis=mybir.AxisListType.X, op=OP.add)
                  nc.scalar.activation(logz[:, r:r + 1], ssum[:], AF.Ln)
                  nc.vector.reciprocal(rsum[:, r:r + 1], ssum[:])

              def p2_ln(r, j):
                  # cached cols: out = ln(exp_saved * (1/sum))
                  s0 = j * LNW
                  stg = sp.tile([128, SUB], f32, tag="stg", name="stg", bufs=4)
                  nc.scalar.activation(
                      stg[:, :LNW], cache[:, s0:s0 + LNW], AF.Ln,
                      scale=rsum[:, r:r + 1])
                  nc.scalar.dma_start(
                      out=out_d[128 * r:128 * (r + 1), s0:s0 + LNW],
                      in_=stg[:, :LNW])

              def p2_mm(r, j, size, pool):
                  # uncached cols: recompute logits, subtract ln(sum)
                  s0 = VCACHE + j * size
                  ss = min(size, V - s0)
                  pj2 = pool.tile([128, size], f32, tag="pj", name="pj2")
                  for v in range(_ceil_div(ss, VT)):
                      v0 = s0 + v * VT
                      vs = min(VT, V - v0)
                      nc.tensor.matmul(
                          out=pj2[:, VT * v:VT * v + vs],
                          lhsT=htabr[:, 128 * r:128 * (r + 1)],
                          rhs=wo_sb[:, v0:v0 + vs],
                          start=True, stop=True,
                      )
                  stg = sp.tile([128, SUB], f32, tag="stg", name="stg", bufs=4)
                  nc.vector.tensor_scalar(
                      out=stg[:, :ss], in0=pj2[:, :ss],
                      scalar1=logz[:, r:r + 1], scalar2=None,
                      op0=OP.subtract)
                  nc.sync.dma_start(
                      out=out_d[128 * r:128 * (r + 1), s0:s0 + ss],
                      in_=stg[:, :ss])

              NLN = VCACHE // LNW                 # 6 ln chunks per tile
              NSO = _ceil_div(V, SUBO)            # 33 overlap p1 chunks
              NRO = _ceil_div(V - VCACHE, SUBO)   # 25 overlap rec chunks
              NSM = _ceil_div(V, SUB)             # 25 main p1 chunks
              NRM = _ceil_div(V - VCACHE, SUB)    # 19 main rec chunks

              # ------------- LSTM + overlapped projection start -------------
              with tc.tile_pool(name="lstm", bufs=3) as lp, \
                   tc.tile_pool(name="lstmpsum", bufs=2, space="PSUM") as lpp, \
                   tc.tile_pool(name="ovlpsum", bufs=2, space="PSUM") as jpo:

                def step(t):
                    gall = lpp.tile([64, 32], f32, tag="gall", name="gall")
                    nc.tensor.matmul(
                        out=gall[:],
                        lhsT=id64[:],
                        rhs=xwall[:, 32 * t:32 * (t + 1)],
                        start=True, stop=False,
                    )
                    for g in range(4):
                        nc.tensor.matmul(
                            out=gall[:, 8 * g:8 * (g + 1)],
                            lhsT=wh_sb[:, 64 * g:64 * (g + 1)],
                            rhs=htab[0:64, 8 * t:8 * (t + 1)],
                            start=False, stop=(g == 3),
                            skip_group_check=True,
                        )
                    sall = lp.tile([64, 32], f32, tag="sall", name="sall")
                    nc.scalar.activation(sall[:], gall[:], AF.Sigmoid)
                    cts = lp.tile([64, 8], f32, tag="cts", name="cts")
                    nc.vector.tensor_scalar(
                        out=cts[:], in0=sall[:, 24:32],
                        scalar1=2.0, scalar2=-1.0, op0=OP.mult, op1=OP.add)
                    t2 = lp.tile([64, 8], f32, tag="t2", name="t2")
                    nc.vector.tensor_tensor(out=t2[:], in0=sall[:, 8:16], in1=cts[:], op=OP.mult)
                    t3 = lp.tile([64, 8], f32, tag="t3", name="t3")
                    nc.vector.tensor_tensor(out=t3[:], in0=sall[:, 0:8], in1=cst[:], op=OP.mult)
                    nc.vector.tensor_tensor(out=cst[:], in0=t2[:], in1=t3[:], op=OP.add)
                    th = lp.tile([64, 8], f32, tag="th", name="th")
                    nc.scalar.activation(th[:], cst[:], AF.Tanh)
                    nc.vector.tensor_tensor(
                        out=htab[0:64, 8 * (t + 1):8 * (t + 2)],
                        in0=th[:], in1=sall[:, 16:24], op=OP.mult)

                for t in range(0, OSTEP):
                    step(t)

                # phase A: steps 80..95 || p1(3)
                copy_htabr(3)
                done = 0
                for i, t in enumerate(range(OSTEP, OSTEP + 16)):
                    step(t)
                    tgt = (i + 1) * NSO // 16
                    while done < tgt:
                        p1_chunk(3, done, SUBO, jpo)
                        done += 1
                p1_finish(3, NSO)

                # phase B: steps 96..111 || p2(3) + p1(4)
                copy_htabr(4)
                for j in range(NLN):      # all cache reads of tile 3 BEFORE
                    p2_ln(3, j)           # tile 4's exp overwrites the cache
                emitted = [0, 0]          # rec(3), p1(4)
                for i, t in enumerate(range(OSTEP + 16, OSTEP + 32)):
                    step(t)
                    for k, total in ((0, NRO), (1, NSO)):
                        tgt = (i + 1) * total // 16
                        while emitted[k] < tgt:
                            jj = emitted[k]
                            if k == 0:
                                p2_mm(3, jj, SUBO, jpo)
                            else:
                                p1_chunk(4, jj, SUBO, jpo)
                            emitted[k] += 1
                p1_finish(4, NSO)

                # phase C: steps 112..126 || p2(4) + p1(2)
                copy_htabr(2)
                for j in range(NLN):
                    p2_ln(4, j)
                emitted = [0, 0]
                nrem = (S - 1) - (OSTEP + 32)
                for i, t in enumerate(range(OSTEP + 32, S - 1)):
                    step(t)
                    for k, total in ((0, NRO), (1, NSO)):
                        tgt = (i + 1) * total // nrem
                        while emitted[k] < tgt:
                            jj = emitted[k]
                            if k == 0:
                                p2_mm(4, jj, SUBO, jpo)
                            else:
                                p1_chunk(2, jj, SUBO, jpo)
                            emitted[k] += 1
                p1_finish(2, NSO)

              # xwall + lstm pools closed; main projection phase
              if "proj" not in phases:
                return nc
              with tc.tile_pool(name="mainpsum", bufs=2, space="PSUM") as jpm:
                PAIRS = [(5, 2), (1, 5), (6, 1), (0, 6), (7, 0), (None, 7)]
                for r1, r2 in PAIRS:
                    if r1 is not None:
                        copy_htabr(r1)
                    for j in range(NLN):
                        p2_ln(r2, j)
                    for c in range(NSM):
                        for j in range(c * NRM // NSM, (c + 1) * NRM // NSM):
                            p2_mm(r2, j, SUB, jpm)
                        if r1 is not None:
                            p1_chunk(r1, c, SUB, jpm)
                    if r1 is not None:
                        p1_finish(r1, NSM)
    return nc


def _prep_shared(inputs, bf16_np):
    """Build the numpy operands shared by all cores."""
    f = lambda k: np.asarray(inputs[k], np.float32)
    Wf1, Wi1, WC1, Wo1 = f("Wf1"), f("Wi1"), f("WC1"), f("Wo1")
    Wf2, Wi2, WC2, Wo2 = f("Wf2"), f("Wi2"), f("WC2"), f("Wo2")

    def rep(w):  # [128,1] -> [128,32] replicated
        return np.tile(w, (1, 32)).astype(np.float32)

    wx = np.concatenate(
        [rep(Wf1[HS:, :]), rep(Wi1[HS:, :]), rep(Wo1[HS:, :]), 2.0 * WC1[HS:, :],
         rep(Wf2[HS:, :]), rep(Wi2[HS:, :]), rep(Wo2[HS:, :]), 2.0 * WC2[HS:, :]],
        axis=1)  # [128, 256]
    # block-diag per gate: [64, 64] block g = [[whf_g, 0], [0, whb_g]]
    whf = np.concatenate(
        [rep(Wf1[:HS, :]), rep(Wi1[:HS, :]), rep(Wo1[:HS, :]), 2.0 * WC1[:HS, :]], axis=1)
    whb = np.concatenate(
        [rep(Wf2[:HS, :]), rep(Wi2[:HS, :]), rep(Wo2[:HS, :]), 2.0 * WC2[:HS, :]], axis=1)
    wh = np.zeros((64, 256), np.float32)
    for g in range(4):
        wh[0:32, 64 * g:64 * g + 32] = whf[:, 32 * g:32 * (g + 1)]
        wh[32:64, 64 * g + 32:64 * (g + 1)] = whb[:, 32 * g:32 * (g + 1)]

    bt = np.zeros((64, 4), np.float32)
    for col, (b1, b2) in enumerate(
            [("bf1", "bf2"), ("bi1", "bi2"), ("bo1", "bo2")]):
        bt[0:32, col] = f(b1)[0]
        bt[32:64, col] = f(b2)[0]
    bt[0:32, 3] = 2.0 * f("bC1")
    bt[32:64, 3] = 2.0 * f("bC2")

    ih = np.zeros((64, 8), np.float32)
    ih[0:32] = np.tile(f("Hf")[:, None], (1, 8))
    ih[32:64] = np.tile(f("Hb")[:, None], (1, 8))
    ic = np.zeros((64, 8), np.float32)
    ic[0:32] = np.tile(f("Cf")[:, None], (1, 8))
    ic[32:64] = np.tile(f("Cb")[:, None], (1, 8))

    wo = np.zeros((65, V), np.float32)
    wo[0:64] = f("Wout")
    wo[64] = f("bout")

    lut = np.ascontiguousarray(f("lookup"))
    return dict(lut=lut, wx=np.ascontiguousarray(wx), wh=np.ascontiguousarray(wh),
                bt=bt, ih=ih, ic=ic,
                wo=np.ascontiguousarray(wo.astype(bf16_np)))


_TRACE = False          # set by test.py to capture an NTFF profile
_LAST_RESULT = None     # BassKernelResults from the most recent run


def kernel(**inputs):
    global _LAST_RESULT
    import concourse.bass as bass
    import concourse.mybir as mybir
    import concourse.tile as tile
    from concourse import bacc
    from concourse.bass_utils import run_bass_kernel_spmd

    nc = bacc.Bacc("TRN2", target_bir_lowering=False)
    _build(nc, tile, mybir, bass)
    nc.compile()

    shared = _prep_shared(inputs, mybir.dt.np(mybir.dt.bfloat16))
    ib = np.asarray(inputs["input_batch"]).astype(np.int32)  # [S, B]

    in_maps = []
    for k in range(NCORES):
        idx_flat = np.ascontiguousarray(ib[:, BL * k:BL * (k + 1)]).reshape(ROWS)
        idx_t = np.ascontiguousarray(idx_flat.reshape(8, 128).T)  # [128, 8]
        in_maps.append(dict(idx=idx_t, **shared))

    res = run_bass_kernel_spmd(nc, in_maps, core_ids=list(range(NCORES)),
                               trace=_TRACE)
    _LAST_RESULT = res
    outs = [r["out"].reshape(S, BL, V) for r in res.results]
    return np.concatenate(outs, axis=1)


if __name__ == "__main__":
    import concourse.bass as bass
    import concourse.mybir as mybir
    import concourse.tile as tile
    from concourse import bacc

    nc = bacc.Bacc("TRN2", target_bir_lowering=False)
    _build(nc, tile, mybir, bass)
    nc.compile()
    print("build ok")


# revision 11
# speedup vs baseline: 1.2938x; 1.0614x over previous
"""BiLSTM + vocab projection + log_softmax Trainium2 kernel.

Strategy (8 NeuronCores, batch-parallel):
  - Shard batch B=64 -> 8 rows per core. LSTM recurrence is per-batch-row,
    so each core runs the full fwd+bwd LSTM over S=128 for its 8 rows.
  - State kept transposed: H^T [32 h-part, 8 b], C^T [32 c-part, 8 b].
    Scalar gates (f,i,o) are broadcast across the 32 c-partitions by
    replicating the gate weight column 32x in the stationary matmul operand,
    so gate*state products are plain elementwise DVE ops (no partition
    broadcasts needed).
  - The per-step H^T write goes directly into a transposed H table
    HtabT [65, 1024] (rows 0:32 fwd h, 32:64 bwd h, row 64 = ones for the
    output bias; col = 8*s + b). Projection lhsT tiles are direct slices.
  - Projection: logits = Hcat @ Wout + bout over V=50257, log_softmax over V.
    Wout_ext [65, V] (row 64 = bout) is loaded ONCE into SBUF as bf16
    (100.5 KB/partition) at kernel start, overlapping the LSTM phase; the
    projection then streams it from SBUF, so HBM traffic is just the
    6.5 MB load + the 206 MB output store (the memory floor).
    Pass 1 computes exp(logits) per chunk via ACT (no max subtraction
    needed: |logits| <= ~12, fp32 safe), accumulating the row sums via
    accum_out and SAVING the exp values for the first VCACHE vocab columns
    as bf16 in SBUF. Pass 2 emits logits - ln(sum) two ways, splitting the
    work across engines so everything hides under the store DMA:
      * cached columns:   ACT  ln(exp_saved * (1/sum))   (no matmul)
      * uncached columns: PE recompute matmul + DVE subtract ln(sum)
    Pass 2 of tile r is interleaved with pass 1 of tile r+1 so the store
    stream never starves.
"""

import numpy as np

V = 50257
E = 128
HS = 32
S = 128
B = 64
NCORES = 8
BL = B // NCORES          # 8 batch rows per core
ROWS = S * BL             # 1024 output rows per core
SUB = 2048                # exp/affine granularity, main phase (4 PSUM banks)
SUBO = 1536               # chunk size while overlapped with the LSTM (3 banks)
VT = 512                  # matmul N tile (one PSUM bank of fp32)
VCACHE = 18432            # leading vocab cols whose exp is cached in SBUF
LNW = 2048                # ln-pass / store width for cached cols
OSTEP = 80                # LSTM step at which tile-3 projection can start


def _ceil_div(a, b):
    return (a + b - 1) // b


def _build(nc, tile, mybir, bass, phases=("pre", "lstm", "proj")):
    from concourse.masks import make_identity

    f32 = mybir.dt.float32
    bf16 = mybir.dt.bfloat16
    AF = mybir.ActivationFunctionType
    OP = mybir.AluOpType

    # ---------------- DRAM I/O ----------------
    idx_d = nc.dram_tensor("idx", [128, 8], mybir.dt.int32, kind="ExternalInput")
    lut_d = nc.dram_tensor("lut", [V, E], f32, kind="ExternalInput")
    wx_d = nc.dram_tensor("wx", [128, 256], f32, kind="ExternalInput")
    wh_d = nc.dram_tensor("wh", [64, 256], bf16, kind="ExternalInput")
    bt_d = nc.dram_tensor("bt", [64, 4], f32, kind="ExternalInput")
    ih_d = nc.dram_tensor("ih", [64, 8], bf16, kind="ExternalInput")
    ic_d = nc.dram_tensor("ic", [64, 8], f32, kind="ExternalInput")
    wo_d = nc.dram_tensor("wo", [65, V], bf16, kind="ExternalInput")
    out_d = nc.dram_tensor("out", [ROWS, V], f32, kind="ExternalOutput")

    nsub = _ceil_div(V, SUB)            # 25 chunks

    with tile.TileContext(nc) as tc:
        with tc.tile_pool(name="persist", bufs=1) as pp:
            # persistent SBUF state
            wo_sb = pp.tile([65, V], bf16)       # resident Wout (+bias row)
            idx_sb = pp.tile([128, 8], mybir.dt.int32)
            wh_sb = pp.tile([64, 256], bf16)     # 4x block-diag [whf_g|whb_g]
            bt_sb = pp.tile([64, 4], f32)
            wx_sb = pp.tile([128, 256], f32)
            id128 = pp.tile([128, 128], f32)
            id64 = pp.tile([64, 64], bf16)
            # time-indexed H table: col-block u = state READ at step u.
            # rows 0:32 fwd (== slot order), rows 32:64 bwd (slot 127-u),
            # row 64 = ones for the output bias.
            htab = pp.tile([65, 8 * S], bf16)
            cst = pp.tile([64, 8], f32)          # C^T state (fwd rows 0:32, bwd 32:64)
            htabr = pp.tile([65, 8 * S], bf16)   # bf16 copy for the projection
            cache = pp.tile([128, VCACHE], bf16) # saved exp(logits), cols 0:VCACHE
            logz = pp.tile([128, 8], f32)        # per row-tile log-partition
            rsum = pp.tile([128, 8], f32)        # per row-tile 1/sum(exp)
            parts = [pp.tile([128, 40], f32, name=f"part{r}") for r in range(8)]

            # Wout load first: no deps, overlaps the whole pre+LSTM phase.
            nc.sync.dma_start(out=wo_sb[:], in_=wo_d[:])
            nc.sync.dma_start(out=idx_sb[:], in_=idx_d[:])
            nc.sync.dma_start(out=wh_sb[:], in_=wh_d[:])
            nc.sync.dma_start(out=bt_sb[:], in_=bt_d[:])
            nc.sync.dma_start(out=wx_sb[:], in_=wx_d[:])
            nc.gpsimd.memset(htab[64:65, :], 1.0)
            make_identity(nc, id128[:])
            make_identity(nc, id64[:])
            # initial states: both directions read col-block 0 at step 0
            nc.sync.dma_start(out=htab[0:64, 0:8], in_=ih_d[:])
            nc.sync.dma_start(out=cst[:], in_=ic_d[:])

            # ---------------- embedding gather + X^T + XW tables ----------------
            if "pre" not in phases:
                return nc
            nc.gpsimd.memset(htabr[64:65, :], 1.0)   # ones row (no htab dep)

            with tc.tile_pool(name="stg", bufs=3) as sp, \
                 tc.tile_pool(name="xw", bufs=1) as xwp:
              xwall = xwp.tile([64, 32 * S], bf16)   # per-step gate pre-acts
              with tc.tile_pool(name="xtb", bufs=1) as xtp, \
                   tc.tile_pool(name="pre", bufs=2) as gp, \
                   tc.tile_pool(name="prepsum", bufs=2, space="PSUM") as gpp:
                xt = xtp.tile([128, ROWS], f32)      # X^T (E on partitions)
                for r in range(8):
                    xg = gp.tile([128, 128], f32, tag="xg", name="xg")
                    nc.gpsimd.indirect_dma_start(
                        out=xg[:],
                        out_offset=None,
                        in_=lut_d[:],
                        in_offset=bass.IndirectOffsetOnAxis(
                            ap=idx_sb[:, r:r + 1], axis=0),
                    )
                    xtp_t = gpp.tile([128, 128], f32, tag="xtp", name="xtp")
                    nc.tensor.transpose(out=xtp_t[:], in_=xg[:], identity=id128[:])
                    nc.vector.tensor_copy(out=xt[:, 128 * r:128 * (r + 1)], in_=xtp_t[:])

                # XW tables: fwd rows hold slot u, bwd rows slot 127-u (the
                # input the bwd cell consumes at step u) via a reversed AP.
                xw_v = xwall[:, :].rearrange("p (s g) -> p s g", g=32)
                xw_r = xw_v[:, ::-1, :]
                for d in range(2):
                    L = 32 * d
                    for g in range(4):
                        for c in range(2):
                            xwp_t = gpp.tile([64, 512], f32, tag="xwp", name="xwp")
                            nc.tensor.matmul(
                                out=xwp_t[L:L + 32, :],
                                lhsT=wx_sb[:, 128 * d + 32 * g:128 * d + 32 * (g + 1)],
                                rhs=xt[:, 512 * c:512 * (c + 1)],
                                start=True, stop=True,
                            )
                            ov = xw_v if d == 0 else xw_r
                            nc.vector.tensor_scalar(
                                out=ov[L:L + 32, 64 * c:64 * (c + 1), 8 * g:8 * (g + 1)],
                                in0=xwp_t[L:L + 32, :].rearrange("p (s b) -> p s b", b=8),
                                scalar1=bt_sb[L:L + 32, g:g + 1],
                                scalar2=None,
                                op0=OP.add,
                            )

              if "lstm" not in phases:
                return nc

              # ---------- shared projection helpers (both phases) ----------
              def copy_htabr(r):
                  # fwd rows are slot==time ordered; bwd rows need reversal
                  nc.vector.tensor_copy(
                      out=htabr[0:32, 128 * r:128 * (r + 1)],
                      in_=htab[0:32, 128 * r:128 * (r + 1)])
                  hb_o = htabr[32:64, :].rearrange("p (s b) -> p s b", b=8)
                  hb_i = htab[32:64, :].rearrange("p (s b) -> p s b", b=8)
                  nc.vector.tensor_copy(
                      out=hb_o[:, 16 * r:16 * (r + 1), :],
                      in_=hb_i[:, ::-1, :][:, 16 * r:16 * (r + 1), :])

              def p1_chunk(r, c, size, pool):
                  s0 = c * size
                  ss = min(size, V - s0)
                  pj = pool.tile([128, size], f32, tag="pj", name="pj")
                  for v in range(_ceil_div(ss, VT)):
                      v0 = s0 + v * VT
                      vs = min(VT, V - v0)
                      nc.tensor.matmul(
                          out=pj[:, VT * v:VT * v + vs],
                          lhsT=htabr[:, 128 * r:128 * (r + 1)],
                          rhs=wo_sb[:, v0:v0 + vs],
                          start=True, stop=True,
                      )
                  if s0 + ss <= VCACHE:
                      eout = cache[:, s0:s0 + ss]
                  else:
                      scr = sp.tile([128, SUB], bf16, tag="scr", name="scr")
                      eout = scr[:, :ss]
                  nc.scalar.activation(
                      eout, pj[:, :ss], AF.Exp,
                      accum_out=parts[r][:, c:c + 1])

              def p1_finish(r, nsubr):
                  ssum = sp.tile([128, 1], f32, tag="ssum", name="ssum")
                  nc.vector.tensor_reduce(
                      out=ssum[:], in_=parts[r][:, :nsubr],
                      axis=mybir.AxisListType.X, op=OP.add)
                  nc.scalar.activation(logz[:, r:r + 1], ssum[:], AF.Ln)
                  nc.vector.reciprocal(rsum[:, r:r + 1], ssum[:])

              def p2_ln(r, j):
                  # cached cols: out = ln(exp_saved * (1/sum))
                  s0 = j * LNW
                  stg = sp.tile([128, SUB], f32, tag="stg", name="stg", bufs=4)
                  nc.scalar.activation(
                      stg[:, :LNW], cache[:, s0:s0 + LNW], AF.Ln,
                      scale=rsum[:, r:r + 1])
                  nc.scalar.dma_start(
                      out=out_d[128 * r:128 * (r + 1), s0:s0 + LNW],
                      in_=stg[:, :LNW])

              def p2_mm(r, j, size, pool):
                  # uncached cols: recompute logits, subtract ln(sum)
                  s0 = VCACHE + j * size
                  ss = min(size, V - s0)
                  pj2 = pool.tile([128, size], f32, tag="pj", name="pj2")
                  for v in range(_ceil_div(ss, VT)):
                      v0 = s0 + v * VT
                      vs = min(VT, V - v0)
                      nc.tensor.matmul(
                          out=pj2[:, VT * v:VT * v + vs],
                          lhsT=htabr[:, 128 * r:128 * (r + 1)],
                          rhs=wo_sb[:, v0:v0 + vs],
                          start=True, stop=True,
                      )
                  stg = sp.tile([128, SUB], f32, tag="stg", name="stg", bufs=4)
                  nc.vector.tensor_scalar(
                      out=stg[:, :ss], in0=pj2[:, :ss],
                      scalar1=logz[:, r:r + 1], scalar2=None,
                      op0=OP.subtract)
                  nc.sync.dma_start(
                      out=out_d[128 * r:128 * (r + 1), s0:s0 + ss],
                      in_=stg[:, :ss])

              NLN = VCACHE // LNW                 # 6 ln chunks per tile
              NSO = _ceil_div(V, SUBO)            # 33 overlap p1 chunks
              NRO = _ceil_div(V - VCACHE, SUBO)   # 25 overlap rec chunks
              NSM = _ceil_div(V, SUB)             # 25 main p1 chunks
              NRM = _ceil_div(V - VCACHE, SUB)    # 19 main rec chunks

              # ------------- LSTM + overlapped projection start -------------
              with tc.tile_pool(name="lstm", bufs=3) as lp, \
                   tc.tile_pool(name="lstmpsum", bufs=2, space="PSUM") as lpp, \
                   tc.tile_pool(name="ovlpsum", bufs=2, space="PSUM") as jpo:

                def step(t):
                    gall = lpp.tile([64, 32], f32, tag="gall", name="gall")
                    nc.tensor.matmul(
                        out=gall[:],
                        lhsT=id64[:],
                        rhs=xwall[:, 32 * t:32 * (t + 1)],
                        start=True, stop=False,
                    )
                    for g in range(4):
                        nc.tensor.matmul(
                            out=gall[:, 8 * g:8 * (g + 1)],
                            lhsT=wh_sb[:, 64 * g:64 * (g + 1)],
                            rhs=htab[0:64, 8 * t:8 * (t + 1)],
                            start=False, stop=(g == 3),
                            skip_group_check=True,
                        )
                    sall = lp.tile([64, 32], f32, tag="sall", name="sall")
                    nc.scalar.activation(sall[:], gall[:], AF.Sigmoid)
                    cts = lp.tile([64, 8], f32, tag="cts", name="cts")
                    nc.vector.tensor_scalar(
                        out=cts[:], in0=sall[:, 24:32],
                        scalar1=2.0, scalar2=-1.0, op0=OP.mult, op1=OP.add)
                    t2 = lp.tile([64, 8], f32, tag="t2", name="t2")
                    nc.vector.tensor_tensor(out=t2[:], in0=sall[:, 8:16], in1=cts[:], op=OP.mult)
                    t3 = lp.tile([64, 8], f32, tag="t3", name="t3")
                    nc.vector.tensor_tensor(out=t3[:], in0=sall[:, 0:8], in1=cst[:], op=OP.mult)
                    nc.vector.tensor_tensor(out=cst[:], in0=t2[:], in1=t3[:], op=OP.add)
                    th = lp.tile([64, 8], f32, tag="th", name="th")
                    nc.scalar.activation(th[:], cst[:], AF.Tanh)
                    nc.vector.tensor_tensor(
                        out=htab[0:64, 8 * (t + 1):8 * (t + 2)],
                        in0=th[:], in1=sall[:, 16:24], op=OP.mult)

                for t in range(0, OSTEP):
                    step(t)

                # phase A: steps 80..95 || p1(3)
                copy_htabr(3)
                done = 0
                for i, t in enumerate(range(OSTEP, OSTEP + 16)):
                    step(t)
                    tgt = (i + 1) * NSO // 16
                    while done < tgt:
                        p1_chunk(3, done, SUBO, jpo)
                        done += 1
                p1_finish(3, NSO)

                # phase B: steps 96..111 || p2(3) + p1(4)
                copy_htabr(4)
                for j in range(NLN):      # all cache reads of tile 3 BEFORE
                    p2_ln(3, j)           # tile 4's exp overwrites the cache
                emitted = [0, 0]          # rec(3), p1(4)
                for i, t in enumerate(range(OSTEP + 16, OSTEP + 32)):
                    step(t)
                    for k, total in ((0, NRO), (1, NSO)):
                        tgt = (i + 1) * total // 16
                        while emitted[k] < tgt:
                            jj = emitted[k]
                            if k == 0:
                                p2_mm(3, jj, SUBO, jpo)
                            else:
                                p1_chunk(4, jj, SUBO, jpo)
                            emitted[k] += 1
                p1_finish(4, NSO)

                # phase C: steps 112..126 || p2(4) + p1(2)
                copy_htabr(2)
                for j in range(NLN):
                    p2_ln(4, j)
                emitted = [0, 0]
                nrem = (S - 1) - (OSTEP + 32)
                for i, t in enumerate(range(OSTEP + 32, S - 1)):
                    step(t)
                    for k, total in ((0, NRO), (1, NSO)):
                        tgt = (i + 1) * total // nrem
                        while emitted[k] < tgt:
                            jj = emitted[k]
                            if k == 0:
                                p2_mm(4, jj, SUBO, jpo)
                            else:
                                p1_chunk(2, jj, SUBO, jpo)
                            emitted[k] += 1
                p1_finish(2, NSO)

              # xwall + lstm pools closed; main projection phase
              if "proj" not in phases:
                return nc
              with tc.tile_pool(name="mainpsum", bufs=2, space="PSUM") as jpm:
                PAIRS = [(5, 2), (1, 5), (6, 1), (0, 6), (7, 0), (None, 7)]
                for r1, r2 in PAIRS:
                    if r1 is not None:
                        copy_htabr(r1)
                    for j in range(NLN):
                        p2_ln(r2, j)
                    for c in range(NSM):
                        for j in range(c * NRM // NSM, (c + 1) * NRM // NSM):
                            p2_mm(r2, j, SUB, jpm)
                        if r1 is not None:
                            p1_chunk(r1, c, SUB, jpm)
                    if r1 is not None:
                        p1_finish(r1, NSM)
    return nc


def _prep_shared(inputs, bf16_np):
    """Build the numpy operands shared by all cores."""
    f = lambda k: np.asarray(inputs[k], np.float32)
    Wf1, Wi1, WC1, Wo1 = f("Wf1"), f("Wi1"), f("WC1"), f("Wo1")
    Wf2, Wi2, WC2, Wo2 = f("Wf2"), f("Wi2"), f("WC2"), f("Wo2")

    def rep(w):  # [128,1] -> [128,32] replicated
        return np.tile(w, (1, 32)).astype(np.float32)

    wx = np.concatenate(
        [rep(Wf1[HS:, :]), rep(Wi1[HS:, :]), rep(Wo1[HS:, :]), 2.0 * WC1[HS:, :],
         rep(Wf2[HS:, :]), rep(Wi2[HS:, :]), rep(Wo2[HS:, :]), 2.0 * WC2[HS:, :]],
        axis=1)  # [128, 256]
    # block-diag per gate: [64, 64] block g = [[whf_g, 0], [0, whb_g]]
    whf = np.concatenate(
        [rep(Wf1[:HS, :]), rep(Wi1[:HS, :]), rep(Wo1[:HS, :]), 2.0 * WC1[:HS, :]], axis=1)
    whb = np.concatenate(
        [rep(Wf2[:HS, :]), rep(Wi2[:HS, :]), rep(Wo2[:HS, :]), 2.0 * WC2[:HS, :]], axis=1)
    wh = np.zeros((64, 256), np.float32)
    for g in range(4):
        wh[0:32, 64 * g:64 * g + 32] = whf[:, 32 * g:32 * (g + 1)]
        wh[32:64, 64 * g + 32:64 * (g + 1)] = whb[:, 32 * g:32 * (g + 1)]

    bt = np.zeros((64, 4), np.float32)
    for col, (b1, b2) in enumerate(
            [("bf1", "bf2"), ("bi1", "bi2"), ("bo1", "bo2")]):
        bt[0:32, col] = f(b1)[0]
        bt[32:64, col] = f(b2)[0]
    bt[0:32, 3] = 2.0 * f("bC1")
    bt[32:64, 3] = 2.0 * f("bC2")

    ih = np.zeros((64, 8), np.float32)
    ih[0:32] = np.tile(f("Hf")[:, None], (1, 8))
    ih[32:64] = np.tile(f("Hb")[:, None], (1, 8))
    ic = np.zeros((64, 8), np.float32)
    ic[0:32] = np.tile(f("Cf")[:, None], (1, 8))
    ic[32:64] = np.tile(f("Cb")[:, None], (1, 8))

    wo = np.zeros((65, V), np.float32)
    wo[0:64] = f("Wout")
    wo[64] = f("bout")

    lut = np.ascontiguousarray(f("lookup"))
    return dict(lut=lut, wx=np.ascontiguousarray(wx),
                wh=np.ascontiguousarray(wh.astype(bf16_np)),
                bt=bt, ih=np.ascontiguousarray(ih.astype(bf16_np)), ic=ic,
                wo=np.ascontiguousarray(wo.astype(bf16_np)))


_TRACE = False          # set by test.py to capture an NTFF profile
_LAST_RESULT = None     # BassKernelResults from the most recent run


def kernel(**inputs):
    global _LAST_RESULT
    import concourse.bass as bass
    import concourse.mybir as mybir
    import concourse.tile as tile
    from concourse import bacc
    from concourse.bass_utils import run_bass_kernel_spmd

    nc = bacc.Bacc("TRN2", target_bir_lowering=False)
    _build(nc, tile, mybir, bass)
    nc.compile()

    shared = _prep_shared(inputs, mybir.dt.np(mybir.dt.bfloat16))
    ib = np.asarray(inputs["input_batch"]).astype(np.int32)  # [S, B]

    in_maps = []
    for k in range(NCORES):
        idx_flat = np.ascontiguousarray(ib[:, BL * k:BL * (k + 1)]).reshape(ROWS)
        idx_t = np.ascontiguousarray(idx_flat.reshape(8, 128).T)  # [128, 8]
        in_maps.append(dict(idx=idx_t, **shared))

    res = run_bass_kernel_spmd(nc, in_maps, core_ids=list(range(NCORES)),
                               trace=_TRACE)
    _LAST_RESULT = res
    outs = [r["out"].reshape(S, BL, V) for r in res.results]
    return np.concatenate(outs, axis=1)


if __name__ == "__main__":
    import concourse.bass as bass
    import concourse.mybir as mybir
    import concourse.tile as tile
    from concourse import bacc

    nc = bacc.Bacc("TRN2", target_bir_lowering=False)
    _build(nc, tile, mybir, bass)
    nc.compile()
    print("build ok")
